# revision 1
# baseline (speedup 1.0000x reference)
"""AudioMamba (4-layer bimamba) forward pass on 8 Trainium2 NeuronCores.

Sharding: batch x d_inner-half.  Core 2b handles (batch b, d_inner[0:512]),
core 2b+1 handles (batch b, d_inner[512:1024]).  Every core computes the
full xc/conv/u/x_proj redundantly (avoids an all-reduce mid-layer); the SSM
(dt, dA, dBu, scan, y) and out_proj run on the core's d_inner half for both
scan directions.  One pairwise AllReduce per layer combines the out_proj
partial sums; the residual stream h stays replicated within each pair.

SPMD note: all 8 cores run one graph, so per-core differences live purely in
the input data.  The full-d_inner e-tile order is host-permuted so that the
core's own half always occupies tiles 0..3.

Layout: features on partitions, sequence (L=256) on the free dim everywhere.
The selective scan uses the DVE TensorTensorScanArith instruction on
(128, 16*256) tiles: the 16 state channels are chained along the free dim
and isolated by zeroing dA at each segment start.  The backward direction
stores its SSM tensors time-reversed (written via negative-stride APs) so
the same ascending scan implements the flipped recurrence.
"""

import numpy as np
import ml_dtypes

BF = ml_dtypes.bfloat16

B, L, D, DI, DIH = 4, 256, 512, 1024, 512
S, R, KCONV, DEPTH, NCLS = 16, 32, 4, 4, 10
P = 128
NKD = D // P          # 4  k-tiles over d_model
NE = DI // P          # 8  e-tiles over full d_inner
NEO = DIH // P        # 4  e-tiles over own half
SEG = L               # 256
BIG = S * SEG         # 4096
EPS = 1e-5

_CACHE = {}


# ----------------------------------------------------------------------------
# host-side weight preparation
# ----------------------------------------------------------------------------

def _prep_core(inp, b, m, use_ladder):
    f32 = np.float32
    moff = m * DIH
    out = {}

    x = np.asarray(inp["x"], f32)
    xr = x[b, 0].reshape(8, 16, 32, 16).transpose(1, 3, 0, 2).reshape(256, 256)
    out["xpatch"] = np.ascontiguousarray(xr.reshape(2, 128, 256)).astype(BF)
    pw = np.asarray(inp["patch_w"], f32).reshape(D, 256)
    out["patch_wT"] = np.ascontiguousarray(pw.T.reshape(2, 128, D)).astype(BF)
    out["patch_b"] = np.ascontiguousarray(
        np.asarray(inp["patch_b"], f32).reshape(NKD, P).T)

    in_proj = np.asarray(inp["in_proj_w"], f32)     # (DEPTH, 2*DI, D)
    norm_w = np.asarray(inp["norm_w"], f32)
    norm_b = np.asarray(inp["norm_b"], f32)
    out_proj = np.asarray(inp["out_proj_w"], f32)   # (DEPTH, D, DI)

    xc_lhsT = np.zeros((DEPTH, P, NKD, NEO, P), BF)
    xc_bias = np.zeros((DEPTH, P, NEO), f32)
    z_lhsT = np.zeros((DEPTH, P, NKD, NEO, P), BF)
    z_bias = np.zeros((DEPTH, P, NEO), f32)
    convd = np.zeros((DEPTH, P, 2, NEO, KCONV, P), BF)
    conv_bias = np.zeros((DEPTH, P, 2, NEO), f32)
    xproj_lhsT = np.zeros((DEPTH, P, 2, NEO, 80), BF)
    dtproj_lhsT = np.zeros((DEPTH, R, 2, NEO, P), BF)
    dt_bias = np.zeros((DEPTH, P, 2, NEO), f32)
    A_cols = np.zeros((DEPTH, P, 2, NEO, S), f32)
    Dd = np.zeros((DEPTH, P, 2, NEO, P), BF)
    outp_lhsT = np.zeros((DEPTH, P, NEO, NKD, P), BF)

    di = np.diag_indices(P)
    for l in range(DEPTH):
        Wp = in_proj[l] * norm_w[l][None, :]
        bp = in_proj[l] @ norm_b[l]
        wxc = Wp[moff:moff + DIH]           # own half of the xc rows
        xc_lhsT[l] = wxc.T.reshape(NKD, P, NEO, P).transpose(1, 0, 2, 3)
        xc_bias[l] = bp[moff:moff + DIH].reshape(NEO, P).T
        wz = Wp[DI + moff: DI + moff + DIH]
        z_lhsT[l] = wz.T.reshape(NKD, P, NEO, P).transpose(1, 0, 2, 3)
        z_bias[l] = bp[DI + moff: DI + moff + DIH].reshape(NEO, P).T

        for d_i, sfx in enumerate(("f", "b")):
            cw = np.asarray(inp[f"conv_w_{sfx}"], f32)[l]     # (DI, K)
            cb = np.asarray(inp[f"conv_b_{sfx}"], f32)[l]
            for et in range(NEO):
                for tap in range(KCONV):
                    v = cw[moff + et * P:moff + (et + 1) * P, tap].astype(BF)
                    convd[l, :, d_i, et, tap, :][di] = v
            conv_bias[l, :, d_i] = cb[moff:moff + DIH].reshape(NEO, P).T
            xw = np.asarray(inp[f"xproj_w_{sfx}"], f32)[l]    # (64, DI)
            xp_h = xw[:, moff:moff + DIH].T.reshape(NEO, P, 64).transpose(1, 0, 2)
            xproj_lhsT[l, :, d_i, :, 0:48] = xp_h[:, :, 0:48]
            xproj_lhsT[l, :, d_i, :, 64:80] = xp_h[:, :, 48:64]
            dtw = np.asarray(inp[f"dtproj_w_{sfx}"], f32)[l]  # (DI, R)
            dts = dtw[moff:moff + DIH]
            dtproj_lhsT[l, :, d_i] = dts.T.reshape(R, NEO, P)
            dt_bias[l, :, d_i] = np.asarray(
                inp[f"dtproj_b_{sfx}"], f32)[l][moff:moff + DIH].reshape(NEO, P).T
            A = -np.exp(np.asarray(inp[f"A_log_{sfx}"], f32)[l])
            A_cols[l, :, d_i] = A[moff:moff + DIH].reshape(NEO, P, S).transpose(1, 0, 2)
            Dv = np.asarray(inp[f"D_{sfx}"], f32)[l][moff:moff + DIH]
            for eo in range(NEO):
                Dd[l, :, d_i, eo, :][di] = Dv[eo * P:(eo + 1) * P].astype(BF)

        Wo = out_proj[l][:, moff:moff + DIH]                  # (512, 512)
        outp_lhsT[l] = Wo.T.reshape(NEO, P, NKD, P).transpose(1, 0, 2, 3)

    out["xc_lhsT"] = np.ascontiguousarray(xc_lhsT)
    out["xc_bias"] = np.ascontiguousarray(xc_bias)
    out["z_lhsT"] = np.ascontiguousarray(z_lhsT)
    out["z_bias"] = np.ascontiguousarray(z_bias)
    out["convd"] = np.ascontiguousarray(convd)
    out["conv_bias"] = np.ascontiguousarray(conv_bias)
    out["xproj_lhsT"] = np.ascontiguousarray(xproj_lhsT)
    out["dtproj_lhsT"] = np.ascontiguousarray(dtproj_lhsT)
    out["dt_bias"] = np.ascontiguousarray(dt_bias)
    out["A_cols"] = np.ascontiguousarray(A_cols)
    out["Dd"] = np.ascontiguousarray(Dd)
    out["outp_lhsT"] = np.ascontiguousarray(outp_lhsT)

    out["normf_w"] = np.ascontiguousarray(
        (np.asarray(inp["normf_w"], f32) / L).reshape(NKD, P).T)
    out["normf_b"] = np.ascontiguousarray(
        np.asarray(inp["normf_b"], f32).reshape(NKD, P).T)
    out["ln_w"] = np.ascontiguousarray(
        np.asarray(inp["ln_w"], f32).reshape(NKD, P).T)
    out["ln_b"] = np.ascontiguousarray(
        np.asarray(inp["ln_b"], f32).reshape(NKD, P).T)
    fc1 = np.asarray(inp["fc1_w"], f32)
    out["fc1_lhsT"] = np.ascontiguousarray(
        fc1.T.reshape(NKD, P, NKD, P).transpose(1, 0, 2, 3)).astype(BF)
    out["fc1_b"] = np.ascontiguousarray(
        np.asarray(inp["fc1_b"], f32).reshape(NKD, P).T)
    fc2 = np.asarray(inp["fc2_w"], f32)
    out["fc2_lhsT"] = np.ascontiguousarray(
        fc2.T.reshape(NKD, P, NCLS).transpose(1, 0, 2)).astype(BF)
    out["fc2_b"] = np.asarray(inp["fc2_b"], f32).reshape(NCLS, 1)
    out["ident"] = np.eye(P, dtype=BF)
    selb = np.zeros((S, S * P), BF)
    for s in range(S):
        selb[s, s * P:(s + 1) * P] = 1
    out["selb"] = selb
    return out


# ----------------------------------------------------------------------------
# device graph
# ----------------------------------------------------------------------------

def _build_graph(use_ladder):
    import concourse.bass as bass
    import concourse.tile as tile
    from concourse import bacc, mybir
    from concourse.tile_rust import add_dep_helper
    from concourse import hw_specs

    # Force exp+ln to resolve to the combined natural_log_exp table set so
    # the dt path (Exp, Ln, Exp back-to-back) doesn't thrash ACT table loads.
    if not getattr(bacc, "_act_tables_patched", False):
        _orig_tables = hw_specs.get_activation_tables

        def _tables(arch):
            t = dict(_orig_tables(arch))
            AF_ = mybir.ActivationFunctionType
            for k in ("exp_and_others", "natural_log", "exp_and_friends"):
                if k in t:
                    t[k] = t[k] - {AF_.Exp, AF_.Ln}
            return t

        bacc.get_activation_tables = _tables
        bacc._act_tables_patched = True

    f32, bf16 = mybir.dt.float32, mybir.dt.bfloat16
    AF = mybir.ActivationFunctionType
    OP = mybir.AluOpType

    nc = bacc.Bacc("TRN2", target_bir_lowering=False)

    def din(name, shape, dtype):
        return nc.dram_tensor(name, list(shape), dtype, kind="ExternalInput")

    xpatch_d = din("xpatch", (2, P, 256), bf16)
    patch_wT_d = din("patch_wT", (2, P, D), bf16)
    patch_b_d = din("patch_b", (P, NKD), f32)
    xc_lhsT_d = din("xc_lhsT", (DEPTH, P, NKD, NEO, P), bf16)
    xc_bias_d = din("xc_bias", (DEPTH, P, NEO), f32)
    z_lhsT_d = din("z_lhsT", (DEPTH, P, NKD, NEO, P), bf16)
    z_bias_d = din("z_bias", (DEPTH, P, NEO), f32)
    convd_d = din("convd", (DEPTH, P, 2, NEO, KCONV, P), bf16)
    conv_bias_d = din("conv_bias", (DEPTH, P, 2, NEO), f32)
    xproj_lhsT_d = din("xproj_lhsT", (DEPTH, P, 2, NEO, 80), bf16)
    dtproj_lhsT_d = din("dtproj_lhsT", (DEPTH, R, 2, NEO, P), bf16)
    dt_bias_d = din("dt_bias", (DEPTH, P, 2, NEO), f32)
    A_cols_d = din("A_cols", (DEPTH, P, 2, NEO, S), f32)
    Dd_d = din("Dd", (DEPTH, P, 2, NEO, P), bf16)
    outp_lhsT_d = din("outp_lhsT", (DEPTH, P, NEO, NKD, P), bf16)
    normf_w_d = din("normf_w", (P, NKD), f32)
    normf_b_d = din("normf_b", (P, NKD), f32)
    ln_w_d = din("ln_w", (P, NKD), f32)
    ln_b_d = din("ln_b", (P, NKD), f32)
    fc1_lhsT_d = din("fc1_lhsT", (P, NKD, NKD, P), bf16)
    fc1_b_d = din("fc1_b", (P, NKD), f32)
    fc2_lhsT_d = din("fc2_lhsT", (P, NKD, NCLS), bf16)
    fc2_b_d = din("fc2_b", (NCLS, 1), f32)
    ident_d = din("ident", (P, P), bf16)
    selb_d = din("selb", (S, S * P), bf16)
    out_d = nc.dram_tensor("out", [NCLS, 1], f32, kind="ExternalOutput")

    def rev2(ap):
        (p0, pc), (fs, fc) = ap.ap
        assert fs == 1, ap.ap
        return bass.AP(tensor=ap.tensor, offset=ap.offset + (fc - 1),
                       ap=[[p0, pc], [-1, fc]])

    def rev3_seg(ap3):
        pdim, sdim, ldim = ap3.ap
        assert ldim[0] == 1
        return bass.AP(tensor=ap3.tensor, offset=ap3.offset + (ldim[1] - 1),
                       ap=[pdim, sdim, [-1, ldim[1]]])

    with tile.TileContext(nc) as tc:
        sb1 = tc.alloc_tile_pool(name="persist", bufs=1)
        hpool = tc.alloc_tile_pool(name="hp", bufs=8)
        wpool = tc.alloc_tile_pool(name="w", bufs=2)
        wpool1 = tc.alloc_tile_pool(name="w1", bufs=1)
        a2 = tc.alloc_tile_pool(name="a2", bufs=2)
        a3 = tc.alloc_tile_pool(name="a3", bufs=3)
        a4 = tc.alloc_tile_pool(name="a4", bufs=4)
        a8 = tc.alloc_tile_pool(name="a8", bufs=8)
        a16 = tc.alloc_tile_pool(name="a16", bufs=16)
        stp = tc.alloc_tile_pool(name="stp", bufs=6)
        scanp = tc.alloc_tile_pool(name="scan", bufs=2)
        bcp = tc.alloc_tile_pool(name="bc", bufs=1)
        pmm = tc.alloc_tile_pool(name="pmm", bufs=4, space="PSUM")
        pbc = tc.alloc_tile_pool(name="pbc", bufs=1, space="PSUM")
        pst = tc.alloc_tile_pool(name="pst", bufs=2, space="PSUM")
        dram = tc.alloc_tile_pool(name="dram", bufs=2, space="DRAM")

        # ---- constants ----
        ones_col = sb1.tile([P, 1], f32)
        nc.vector.memset(ones_col[:], 1.0)
        ones_col_bf = sb1.tile([P, 1], bf16)
        nc.vector.memset(ones_col_bf[:], 1.0)
        ones_row = sb1.tile([1, P], f32)
        nc.vector.memset(ones_row[:], 1.0)
        eps_t = sb1.tile([1, 1], f32)
        nc.vector.memset(eps_t[:], EPS)
        ident = sb1.tile([P, P], bf16)
        nc.sync.dma_start(ident[:], ident_d[:])
        selb = sb1.tile([S, S * P], bf16)
        nc.sync.dma_start(selb[:], selb_d[:])

        patch_b_t = sb1.tile([P, NKD], f32)
        nc.sync.dma_start(patch_b_t[:], patch_b_d[:])
        normf_w_t = sb1.tile([P, NKD], f32)
        nc.sync.dma_start(normf_w_t[:], normf_w_d[:])
        normf_b_t = sb1.tile([P, NKD], f32)
        nc.sync.dma_start(normf_b_t[:], normf_b_d[:])
        ln_w_t = sb1.tile([P, NKD], f32)
        nc.sync.dma_start(ln_w_t[:], ln_w_d[:])
        ln_b_t = sb1.tile([P, NKD], f32)
        nc.sync.dma_start(ln_b_t[:], ln_b_d[:])
        fc1w = sb1.tile([P, NKD * NKD * P], bf16)
        nc.sync.dma_start(fc1w[:], fc1_lhsT_d[:].rearrange("p a b m -> p (a b m)"))
        fc1_b_t = sb1.tile([P, NKD], f32)
        nc.sync.dma_start(fc1_b_t[:], fc1_b_d[:])
        fc2w = sb1.tile([P, NKD * NCLS], bf16)
        nc.sync.dma_start(fc2w[:], fc2_lhsT_d[:].rearrange("p a m -> p (a m)"))
        fc2_b_t = sb1.tile([NCLS, 1], f32)
        nc.sync.dma_start(fc2_b_t[:], fc2_b_d[:])

        # ---- warm up the collective trigger path (first CC pays ~11us
        #      of one-time setup; absorb it here where nothing waits) ----
        warm_s = a2.tile([P, 16], f32, tag="warm_s")
        nc.vector.memset(warm_s[:], 0.0)
        warm_in = dram.tile([P, 16], f32, tag="warm_in")
        warm_out = dram.tile([P, 16], f32, tag="warm_out")
        nc.sync.dma_start(warm_in[:], warm_s[:])
        nc.gpsimd.collective_compute(
            "AllReduce", OP.add,
            replica_groups=[[0, 1], [2, 3], [4, 5], [6, 7]],
            ins=[warm_in.opt()], outs=[warm_out.opt()])

        # ---- patch embed -> h (4 x (128 d, 256 l) f32) ----
        h = []
        xpt = [a2.tile([P, 256], bf16, tag="xpatch", name=f"xpt{i}") for i in range(2)]
        for kt in range(2):
            nc.sync.dma_start(xpt[kt][:], xpatch_d[kt])
        pwt = [a2.tile([P, D], bf16, tag="pwT", name=f"pwt{i}") for i in range(2)]
        for kt in range(2):
            nc.sync.dma_start(pwt[kt][:], patch_wT_d[kt])
        for mt in range(NKD):
            ps = pmm.tile([P, SEG], f32, tag="mm")
            for kt in range(2):
                nc.tensor.matmul(ps[:], pwt[kt][:, mt * P:(mt + 1) * P], xpt[kt][:],
                                 start=(kt == 0), stop=(kt == 1))
            t = hpool.tile([P, SEG], f32, tag="h")
            nc.scalar.activation(t[:], ps[:], AF.Identity,
                                 bias=patch_b_t[:, mt:mt + 1])
            h.append(t)

        # ---- layernorm over d (partition dim) ----
        def layer_norm(htiles):
            sums = pst.tile([1, SEG], f32, tag="st")
            for kt in range(NKD):
                nc.tensor.matmul(sums[:], ones_col[:], htiles[kt][:],
                                 start=(kt == 0), stop=(kt == NKD - 1))
            hsq = []
            for kt in range(NKD):
                t = a4.tile([P, SEG], bf16, tag="hsq")
                nc.scalar.activation(t[:], htiles[kt][:], AF.Square)
                hsq.append(t)
            ssq = pst.tile([1, SEG], f32, tag="st")
            for kt in range(NKD):
                nc.tensor.matmul(ssq[:], ones_col_bf[:], hsq[kt][:],
                                 start=(kt == 0), stop=(kt == NKD - 1))
            mean = stp.tile([1, SEG], f32, tag="stat")
            nc.scalar.mul(mean[:], sums[:], 1.0 / D)
            msq = stp.tile([1, SEG], f32, tag="stat")
            nc.scalar.mul(msq[:], ssq[:], 1.0 / D)
            m2 = stp.tile([1, SEG], f32, tag="stat")
            nc.vector.tensor_mul(m2[:], mean[:], mean[:])
            var = stp.tile([1, SEG], f32, tag="stat")
            nc.vector.tensor_sub(var[:], msq[:], m2[:])
            lnv = stp.tile([1, SEG], f32, tag="stat")
            nc.scalar.activation(lnv[:], var[:], AF.Ln, bias=eps_t[:1, :])
            rstd = stp.tile([1, SEG], f32, tag="stat")
            nc.scalar.activation(rstd[:], lnv[:], AF.Exp, scale=-0.5)
            mean_b = pst.tile([P, SEG], f32, tag="st")
            nc.tensor.matmul(mean_b[:], ones_row[:], mean[:], start=True, stop=True)
            rstd_b = pst.tile([P, SEG], f32, tag="st")
            nc.tensor.matmul(rstd_b[:], ones_row[:], rstd[:], start=True, stop=True)
            rstd_sb = a2.tile([P, SEG], bf16, tag="rstd")
            nc.scalar.copy(rstd_sb[:], rstd_b[:])
            xn = []
            for kt in range(NKD):
                t0 = a2.tile([P, SEG], bf16, tag="xn0")
                nc.vector.tensor_sub(t0[:], htiles[kt][:], mean_b[:])
                t1 = a4.tile([P, SEG], bf16, tag="xn")
                nc.vector.tensor_mul(t1[:], t0[:], rstd_sb[:])
                xn.append(t1)
            return xn

        # ---- layers ----
        for l in range(DEPTH):
            xcw = wpool.tile([P, NKD * NEO * P], bf16, tag="xcw")
            nc.sync.dma_start(xcw[:], xc_lhsT_d[l].rearrange("p a b m -> p (a b m)"))
            xcw_v = xcw[:].rearrange("p (a b m) -> p a b m", a=NKD, b=NEO)
            zw = wpool.tile([P, NKD * NEO * P], bf16, tag="zw")
            nc.sync.dma_start(zw[:], z_lhsT_d[l].rearrange("p a b m -> p (a b m)"))
            zw_v = zw[:].rearrange("p (a b m) -> p a b m", a=NKD, b=NEO)
            cvw = wpool1.tile([P, 2 * NEO * KCONV * P], bf16, tag="cvw")
            nc.sync.dma_start(cvw[:], convd_d[l].rearrange("p a b c m -> p (a b c m)"))
            cvw_v = cvw[:].rearrange("p (a b c m) -> p a b c m", a=2, b=NEO, c=KCONV)
            xpw = wpool.tile([P, 2 * NEO * 80], bf16, tag="xpw")
            nc.sync.dma_start(xpw[:], xproj_lhsT_d[l].rearrange("p a b m -> p (a b m)"))
            xpw_v = xpw[:].rearrange("p (a b m) -> p a b m", a=2, b=NEO)
            dtw = wpool.tile([R, 2 * NEO * P], bf16, tag="dtw")
            nc.sync.dma_start(dtw[:], dtproj_lhsT_d[l].rearrange("p a b m -> p (a b m)"))
            dtw_v = dtw[:].rearrange("p (a b m) -> p a b m", a=2, b=NEO)
            ddw = wpool.tile([P, 2 * NEO * P], bf16, tag="ddw")
            nc.sync.dma_start(ddw[:], Dd_d[l].rearrange("p a b m -> p (a b m)"))
            ddw_v = ddw[:].rearrange("p (a b m) -> p a b m", a=2, b=NEO)
            opw = wpool.tile([P, NEO * NKD * P], bf16, tag="opw")
            nc.sync.dma_start(opw[:], outp_lhsT_d[l].rearrange("p a b m -> p (a b m)"))
            opw_v = opw[:].rearrange("p (a b m) -> p a b m", a=NEO, b=NKD)
            xcb = wpool.tile([P, NEO], f32, tag="xcb")
            nc.sync.dma_start(xcb[:], xc_bias_d[l])
            zb = wpool.tile([P, NEO], f32, tag="zb")
            nc.sync.dma_start(zb[:], z_bias_d[l])
            cvb = wpool.tile([P, 2 * NEO], f32, tag="cvb")
            nc.sync.dma_start(cvb[:], conv_bias_d[l].rearrange("p a b -> p (a b)"))
            dtb = wpool.tile([P, 2 * NEO], f32, tag="dtb")
            nc.sync.dma_start(dtb[:], dt_bias_d[l].rearrange("p a b -> p (a b)"))
            act_A = None
            if not use_ladder:
                act_A = wpool.tile([P, 2 * NEO * S], f32, tag="acols")
                nc.sync.dma_start(act_A[:],
                                  A_cols_d[l].rearrange("p a b s -> p (a b s)"))

            xn = layer_norm(h)

            # -- in_proj --
            xc_pad = []
            for et in range(NEO):
                ps = pmm.tile([P, SEG], f32, tag="mm")
                for kt in range(NKD):
                    nc.tensor.matmul(ps[:], xcw_v[:, kt, et, :], xn[kt][:],
                                     start=(kt == 0), stop=(kt == NKD - 1))
                t = a8.tile([P, SEG + 6], bf16, tag="xcpad")
                nc.vector.memset(t[:, 0:3], 0.0)
                nc.vector.memset(t[:, SEG + 3:SEG + 6], 0.0)
                nc.scalar.activation(t[:, 3:SEG + 3], ps[:], AF.Identity,
                                     bias=xcb[:, et:et + 1])
                xc_pad.append(t)
            g = []
            silu_insts = []
            for eo in range(NEO):
                ps = pmm.tile([P, SEG], f32, tag="mm")
                for kt in range(NKD):
                    nc.tensor.matmul(ps[:], zw_v[:, kt, eo, :], xn[kt][:],
                                     start=(kt == 0), stop=(kt == NKD - 1))
                t = a4.tile([P, SEG], bf16, tag="g")
                si = nc.scalar.activation(t[:], ps[:], AF.Silu,
                                          bias=zb[:, eo:eo + 1])
                silu_insts.append(si)
                g.append(t)

            # -- conv + silu -> u --
            u = [[None] * NEO for _ in range(2)]
            for d_i in range(2):
                for et in range(NEO):
                    ps = pmm.tile([P, SEG], f32, tag="mm")
                    for tap in range(KCONV):
                        o = tap if d_i == 0 else 6 - tap
                        nc.tensor.matmul(ps[:], cvw_v[:, d_i, et, tap, :],
                                         xc_pad[et][:, o:o + SEG],
                                         start=(tap == 0), stop=(tap == KCONV - 1))
                    t = a16.tile([P, SEG], bf16, tag="u")
                    nc.scalar.activation(
                        t[:], ps[:], AF.Silu,
                        bias=cvb[:, d_i * NEO + et:d_i * NEO + et + 1])
                    u[d_i][et] = t

            # -- x_proj partial (own half) -> [dtr|B|pad|C] (80 rows);
            #    one pairwise CC per direction (fwd CC overlaps bwd conv) --
            xdblr = []
            xbouts = []
            for d_i in range(2):
                xbin = dram.tile([80, SEG], bf16, tag="xbin", name=f"xbin{d_i}")
                xbout = dram.tile([80, SEG], bf16, tag="xbout", name=f"xbout{d_i}")
                xbouts.append(xbout)
                ps1 = pst.tile([80, SEG], f32, tag="st")
                for kt in range(NEO):
                    nc.tensor.matmul(ps1[:], xpw_v[:, d_i, kt, :], u[d_i][kt][:],
                                     start=(kt == 0), stop=(kt == NEO - 1))
                t = a2.tile([80, SEG], bf16, tag="xpp", name=f"xpp{d_i}")
                nc.scalar.copy(t[:], ps1[:])
                nc.sync.dma_start(xbin[:], t[:])
                nc.gpsimd.collective_compute(
                    "AllReduce", OP.add,
                    replica_groups=[[0, 1], [2, 3], [4, 5], [6, 7]],
                    ins=[xbin.opt()], outs=[xbout.opt()])
                tr = a2.tile([80, SEG], bf16, tag="xdblr", name=f"xdblr{d_i}")
                nc.sync.dma_start(tr[:], xbout[:])
                xdblr.append(tr)
            dtr = [xdblr[0][0:R, :], xdblr[1][0:R, :]]
            Bm, Cm = [], []
            for d_i in range(2):
                tb = a2.tile([S, SEG], bf16, tag="Bm", name=f"Bm{d_i}")
                nc.scalar.copy(tb[:], xdblr[d_i][32:48, :])
                Bm.append(tb)
                tcm = a2.tile([S, SEG], bf16, tag="Cm", name=f"Cm{d_i}")
                nc.scalar.copy(tcm[:], xdblr[d_i][64:80, :])
                Cm.append(tcm)

            # -- B/C broadcast tiles (128, 4096); bwd stored time-reversed --
            bc_tiles = {}
            for d_i in range(2):
                for nm, src_t in (("B", Bm[d_i]), ("C", Cm[d_i])):
                    big = bcp.tile([P, BIG], bf16, tag=nm)
                    for chunk in range(4):
                        ps = pbc.tile([P, 4 * SEG], f32, tag="bc")
                        for j in range(4):
                            s = chunk * 4 + j
                            nc.tensor.matmul(ps[:, j * SEG:(j + 1) * SEG],
                                             selb[:, s * P:(s + 1) * P],
                                             src_t[:],
                                             start=True, stop=True)
                        dst = big[:, chunk * 4 * SEG:(chunk + 1) * 4 * SEG]
                        if d_i == 1:
                            dst = rev3_seg(dst.rearrange("p (s l) -> p s l", s=4))
                            srcp = ps[:].rearrange("p (s l) -> p s l", s=4)
                        else:
                            srcp = ps[:]
                        nc.scalar.copy(dst, srcp)
                    bc_tiles[(d_i, nm)] = big

            # -- SSM per unit (direction, own-half e-tile) --
            y = [[None] * NEO for _ in range(2)]
            for d_i in range(2):
                for eo in range(NEO):
                    gidx = d_i * NEO + eo
                    eng = nc.gpsimd if gidx >= 6 else nc.vector
                    uo = u[d_i][eo]           # own half == global tiles 0..3

                    ps = pmm.tile([P, SEG], f32, tag="mm")
                    nc.tensor.matmul(ps[:], dtw_v[:, d_i, eo, :], dtr[d_i],
                                     start=True, stop=True)
                    e_t = a3.tile([P, SEG], f32, tag="edt")
                    ei = nc.scalar.activation(
                        e_t[:], ps[:], AF.Exp,
                        bias=dtb[:, d_i * NEO + eo:d_i * NEO + eo + 1])
                    dt_t = a3.tile([P, SEG], bf16, tag="dt")
                    li = nc.scalar.activation(dt_t[:], e_t[:], AF.Ln, bias=1.0)
                    for zi in silu_insts:
                        add_dep_helper(ei.ins, zi.ins, sync=False,
                                       reason="act-table grouping")
                        add_dep_helper(li.ins, zi.ins, sync=False,
                                       reason="act-table grouping")

                    dA = scanp.tile([P, BIG], bf16, tag="dA", bufs=3)
                    seg0 = dA[:, 0:SEG]
                    if d_i == 1:
                        seg0 = rev2(seg0)
                    if use_ladder:
                        qi = nc.scalar.activation(seg0, dt_t[:], AF.Exp,
                                                  scale=-1.0)
                        for zi in silu_insts:
                            add_dep_helper(qi.ins, zi.ins, sync=False,
                                           reason="act-table grouping")
                        if eng is nc.vector:
                            # q^2, then pair-width chain x [q^2|q^2] at 2x
                            eng.tensor_mul(dA[:, SEG:2 * SEG],
                                           dA[:, 0:SEG], dA[:, 0:SEG])
                            q2d = a3.tile([P, 2 * SEG], bf16, tag="q2d")
                            eng.tensor_copy(q2d[:, 0:SEG], dA[:, SEG:2 * SEG])
                            eng.tensor_copy(q2d[:, SEG:2 * SEG],
                                            dA[:, SEG:2 * SEG])
                            for s2 in range(2, S, 2):
                                eng.tensor_mul(
                                    dA[:, s2 * SEG:(s2 + 2) * SEG],
                                    dA[:, (s2 - 2) * SEG:s2 * SEG],
                                    q2d[:])
                        else:
                            for s in range(1, S):
                                eng.tensor_mul(dA[:, s * SEG:(s + 1) * SEG],
                                               dA[:, (s - 1) * SEG:s * SEG],
                                               dA[:, 0:SEG])
                    else:
                        base = (d_i * NEO + eo) * S
                        for s in range(S):
                            segs = dA[:, s * SEG:(s + 1) * SEG]
                            if d_i == 1:
                                segs = rev2(segs)
                            ai = nc.scalar.activation(
                                segs, dt_t[:], AF.Exp,
                                scale=act_A[:, base + s:base + s + 1])
                            for zi in silu_insts:
                                add_dep_helper(ai.ins, zi.ins, sync=False,
                                               reason="act-table grouping")

                    dtu = a3.tile([P, SEG], bf16, tag="dtu")
                    nc.vector.tensor_mul(dtu[:], dt_t[:], uo[:])
                    if d_i == 1:
                        dtu_r = a3.tile([P, SEG], bf16, tag="dtur")
                        nc.vector.tensor_copy(rev2(dtu_r[:]), dtu[:])
                        dtu_use = dtu_r
                    else:
                        dtu_use = dtu

                    dBu = scanp.tile([P, BIG], bf16, tag="dBu", bufs=3)
                    Bb = bc_tiles[(d_i, "B")]
                    if eng is nc.vector:
                        # 16 per-segment multiplies run in the DVE 2x mode
                        for s in range(S):
                            eng.tensor_mul(dBu[:, s * SEG:(s + 1) * SEG],
                                           dtu_use[:],
                                           Bb[:, s * SEG:(s + 1) * SEG])
                    else:
                        d_ap = dtu_use[:]
                        drep = bass.AP(tensor=d_ap.tensor, offset=d_ap.offset,
                                       ap=[d_ap.ap[0], [0, S], [1, SEG]])
                        eng.tensor_tensor(
                            dBu[:].rearrange("p (s l) -> p s l", s=S),
                            drep,
                            Bb[:].rearrange("p (s l) -> p s l", s=S), OP.mult)

                    dAr = dA[:].rearrange("p (s l) -> p s l", s=S)
                    nc.vector.memset(dAr[:, :, 0:1], 0.0)
                    hs = scanp.tile([P, BIG], bf16, tag="hs", bufs=3)
                    nc.vector.tensor_tensor_scan(hs[:], dA[:], dBu[:], 0.0,
                                                 OP.mult, OP.add)

                    yc = scanp.tile([P, BIG], bf16, tag="dBu", bufs=3)
                    Cb = bc_tiles[(d_i, "C")]
                    for s2 in range(0, S, 2):
                        lo, hi = s2 * SEG, (s2 + 2) * SEG
                        if d_i == 0:
                            nc.vector.tensor_tensor(
                                yc[:, lo:hi], hs[:, lo:hi], Cb[:, lo:hi],
                                OP.mult)
                        else:
                            dv = yc[:, lo:hi].rearrange(
                                "p (s l) -> p s l", s=2)
                            nc.vector.tensor_tensor(
                                rev3_seg(dv),
                                hs[:, lo:hi].rearrange("p (s l) -> p s l", s=2),
                                Cb[:, lo:hi].rearrange("p (s l) -> p s l", s=2),
                                OP.mult)

                    psy = pmm.tile([P, SEG], f32, tag="mm")
                    for s in range(S):
                        nc.tensor.matmul(psy[:], ident[:],
                                         yc[:, s * SEG:(s + 1) * SEG],
                                         start=(s == 0), stop=False)
                    nc.tensor.matmul(psy[:], ddw_v[:, d_i, eo, :], uo[:],
                                     start=False, stop=True)
                    yt = a8.tile([P, SEG], bf16, tag="y")
                    nc.scalar.copy(yt[:], psy[:])
                    y[d_i][eo] = yt

            # -- gate + out_proj partial --
            yg = []
            for eo in range(NEO):
                ysum = a2.tile([P, SEG], bf16, tag="ysum")
                nc.vector.tensor_add(ysum[:], y[0][eo][:], y[1][eo][:])
                t = a4.tile([P, SEG], bf16, tag="yg")
                nc.vector.tensor_mul(t[:], ysum[:], g[eo][:])
                yg.append(t)
            ocs = []
            for mt in range(NKD):
                ps = pmm.tile([P, SEG], f32, tag="mm")
                for kt in range(NEO):
                    nc.tensor.matmul(ps[:], opw_v[:, kt, mt, :], yg[kt][:],
                                     start=(kt == 0), stop=(kt == NEO - 1))
                t = a4.tile([P, SEG], bf16, tag="oc")
                nc.scalar.copy(t[:], ps[:])
                ocs.append(t)

            # -- pairwise AllReduce; residual add --
            bin_ = dram.tile([D, SEG], bf16, tag="bin")
            bout = dram.tile([D, SEG], bf16, tag="bout")
            for mt in range(NKD):
                nc.sync.dma_start(bin_[mt * P:(mt + 1) * P, :], ocs[mt][:])
            nc.gpsimd.collective_compute(
                "AllReduce", OP.add,
                replica_groups=[[0, 1], [2, 3], [4, 5], [6, 7]],
                ins=[bin_.opt()], outs=[bout.opt()])
            h_new = []
            for mt in range(NKD):
                t = a4.tile([P, SEG], bf16, tag="osum")
                nc.sync.dma_start(t[:], bout[mt * P:(mt + 1) * P, :])
                hn = hpool.tile([P, SEG], f32, tag="h")
                nc.vector.tensor_add(hn[:], h[mt][:], t[:])
                h_new.append(hn)
            h = h_new

        # ---- final norm + mean pool + classifier ----
        xnf = layer_norm(h)
        feat = []
        for kt in range(NKD):
            t = a4.tile([P, 1], f32, tag="feat")
            nc.vector.tensor_reduce(t[:], xnf[kt][:], mybir.AxisListType.X, OP.add)
            t2 = a4.tile([P, 1], f32, tag="feat2")
            nc.vector.tensor_scalar(t2[:], t[:], normf_w_t[:, kt:kt + 1],
                                    normf_b_t[:, kt:kt + 1], OP.mult, OP.add)
            feat.append(t2)
        psum1 = pst.tile([1, 1], f32, tag="st")
        for kt in range(NKD):
            nc.tensor.matmul(psum1[:], ones_col[:], feat[kt][:],
                             start=(kt == 0), stop=(kt == NKD - 1))
        fsq = []
        for kt in range(NKD):
            t = a4.tile([P, 1], f32, tag="fsq")
            nc.scalar.activation(t[:], feat[kt][:], AF.Square)
            fsq.append(t)
        psumq = pst.tile([1, 1], f32, tag="st")
        for kt in range(NKD):
            nc.tensor.matmul(psumq[:], ones_col[:], fsq[kt][:],
                             start=(kt == 0), stop=(kt == NKD - 1))
        meanc = stp.tile([1, 1], f32, tag="sc")
        nc.scalar.mul(meanc[:], psum1[:], 1.0 / D)
        msqc = stp.tile([1, 1], f32, tag="sc")
        nc.scalar.mul(msqc[:], psumq[:], 1.0 / D)
        m2c = stp.tile([1, 1], f32, tag="sc")
        nc.vector.tensor_mul(m2c[:], meanc[:], meanc[:])
        varc = stp.tile([1, 1], f32, tag="sc")
        nc.vector.tensor_sub(varc[:], msqc[:], m2c[:])
        lnvc = stp.tile([1, 1], f32, tag="sc")
        nc.scalar.activation(lnvc[:], varc[:], AF.Ln, bias=eps_t[:])
        rstdc = stp.tile([1, 1], f32, tag="sc")
        nc.scalar.activation(rstdc[:], lnvc[:], AF.Exp, scale=-0.5)
        mb = pst.tile([P, 1], f32, tag="st")
        nc.tensor.matmul(mb[:], ones_row[:], meanc[:], start=True, stop=True)
        rb = pst.tile([P, 1], f32, tag="st")
        nc.tensor.matmul(rb[:], ones_row[:], rstdc[:], start=True, stop=True)
        mb_s = a2.tile([P, 1], f32, tag="mb")
        nc.scalar.copy(mb_s[:], mb[:])
        rb_s = a2.tile([P, 1], f32, tag="rb")
        nc.scalar.copy(rb_s[:], rb[:])
        cvec = []
        for kt in range(NKD):
            t0 = a4.tile([P, 1], f32, tag="c0")
            nc.vector.tensor_sub(t0[:], feat[kt][:], mb_s[:])
            t1 = a4.tile([P, 1], f32, tag="c1")
            nc.vector.tensor_mul(t1[:], t0[:], rb_s[:])
            t2 = a4.tile([P, 1], bf16, tag="c2")
            nc.vector.tensor_scalar(t2[:], t1[:], ln_w_t[:, kt:kt + 1],
                                    ln_b_t[:, kt:kt + 1], OP.mult, OP.add)
            cvec.append(t2)
        fc1w_v = fc1w[:].rearrange("p (a b m) -> p a b m", a=NKD, b=NKD)
        r1 = []
        for mt in range(NKD):
            ps = pst.tile([P, 1], f32, tag="st")
            for kt in range(NKD):
                nc.tensor.matmul(ps[:], fc1w_v[:, kt, mt, :], cvec[kt][:],
                                 start=(kt == 0), stop=(kt == NKD - 1))
            t = a4.tile([P, 1], bf16, tag="r1")
            nc.scalar.activation(t[:], ps[:], AF.Relu, bias=fc1_b_t[:, mt:mt + 1])
            r1.append(t)
        fc2w_v = fc2w[:].rearrange("p (a m) -> p a m", a=NKD)
        ps2 = pst.tile([NCLS, 1], f32, tag="st")
        for kt in range(NKD):
            nc.tensor.matmul(ps2[:], fc2w_v[:, kt, :], r1[kt][:],
                             start=(kt == 0), stop=(kt == NKD - 1))
        logits = a2.tile([NCLS, 1], f32, tag="logits")
        nc.scalar.activation(logits[:], ps2[:], AF.Identity, bias=fc2_b_t[:])
        nc.sync.dma_start(out_d[:], logits[:])

        for _pool in (dram, pst, pbc, pmm, bcp, scanp, stp, a16, a8, a4, a3,
                      a2, wpool1, wpool, hpool, sb1):
            _pool.release()

    nc.finalize()
    return nc


# ----------------------------------------------------------------------------
# entry point
# ----------------------------------------------------------------------------

def kernel(**inputs):
    from concourse.bass_utils import run_bass_kernel_spmd

    f32 = np.float32
    ok = True
    for sfx in ("f", "b"):
        A = -np.exp(np.asarray(inputs[f"A_log_{sfx}"], f32))
        ok = ok and np.allclose(A, -np.arange(1, S + 1, dtype=f32), atol=1e-4)
    use_ladder = bool(ok)

    key = ("graph", use_ladder)
    if key not in _CACHE:
        _CACHE[key] = _build_graph(use_ladder)
    nc = _CACHE[key]

    in_maps = [_prep_core(inputs, c // 2, c % 2, use_ladder) for c in range(8)]
    res = run_bass_kernel_spmd(nc, in_maps, core_ids=list(range(8)))
    outs = res.results
    logits = np.stack([outs[2 * b]["out"][:, 0] for b in range(B)], axis=0)
    return logits.astype(np.float32)

